# revision 4
# baseline (speedup 1.0000x reference)
"""Fused Trainium2 Bass kernel for nn_Attention_46901042872659.

Single software-pipelined pass per batch element, all weights resident
in SBUF (bf16), zero DRAM scratch traffic. GEMM runs two iterations
ahead of attention:

  pre:   qk-GEMM(0), qk-GEMM(1), v-GEMM(0), v-GEMM(1)
  iter b: attn(b) [proj(b-1) groups interleaved at odd heads]
          GEMM(b+2)
  tail:  proj(7)

wqk columns are host-reordered into head-pair groups [q_h0|q_h1],
[k_h0|k_h1], ... so each qkv-GEMM psum group is pure-q or pure-k across
all 128 partitions; head h's QK matmul then reads both k (lhsT) and q
(rhs) at base partition (h%2)*64, satisfying the equal-base-partition
rule without splitting the GEMM (which would double its moving passes).

Per head: s = k^T q -> exp on ACT (bf16) -> *exp(bias) on Pool (the
softmax bias add becomes a multiply after host-side exp); rowsum via
6.0-valued [128,128] ones matmul (PE; output replicated across
partitions = free broadcast); reciprocal on DVE; O = v^T es (PE);
y0 = O*rcp (DVE psum drain); hardswish = y0*min(Relu(6*y0+3), 6)
(ACT + Pool + Pool), with the 1/6 folded into the 6.0 ones vector.

Data-parallel over batch across 8 cores (8 batch elems each).
"""
import numpy as np
import ml_dtypes
from contextlib import ExitStack

import concourse.bass as bass
import concourse.tile as tile
from concourse import bacc, bass_isa, mybir
from concourse.bass_utils import run_bass_kernel_spmd

B, SEQ, DIM = 64, 256, 768
HEADS, KD, DV = 12, 64, 256
H = 4608
DH = 3072
EPS = 1e-5
SCALE = KD ** -0.5
NCORES = 8
BPC = B // NCORES
T = BPC * SEQ
F32 = mybir.dt.float32
F32R = mybir.dt.float32r
BF16 = mybir.dt.bfloat16
ADD = mybir.AluOpType.add
MULT = mybir.AluOpType.mult
EXP = mybir.ActivationFunctionType.Exp
RELU = mybir.ActivationFunctionType.Relu


def _emit(tc, nc, x_t, wqk_t, wv_t, wp_t, bqk, bv, ebs, pb, y_t):
    with ExitStack() as ctx:
        res = ctx.enter_context(tc.tile_pool(name="res", bufs=1))
        xio = ctx.enter_context(tc.tile_pool(name="xio", bufs=2))
        qkio = ctx.enter_context(tc.tile_pool(name="qkio", bufs=2))
        vio = ctx.enter_context(tc.tile_pool(name="vio", bufs=2))
        work = ctx.enter_context(tc.tile_pool(name="work", bufs=2))
        hb = ctx.enter_context(tc.tile_pool(name="hb", bufs=2))
        yio = ctx.enter_context(tc.tile_pool(name="yio", bufs=1))
        so_p = ctx.enter_context(tc.tile_pool(name="sop", bufs=1, space="PSUM"))
        vps_p = ctx.enter_context(tc.tile_pool(name="vpsp", bufs=1, space="PSUM"))
        qps_p = ctx.enter_context(tc.tile_pool(name="qpsp", bufs=1,
                                               space="PSUM"))
        rs_p = ctx.enter_context(tc.tile_pool(name="rsp", bufs=1,
                                              space="PSUM"))
        py_p = ctx.enter_context(tc.tile_pool(name="pyp", bufs=1,
                                              space="PSUM"))

        wqk_sb = res.tile([128, 6, 1536], BF16)
        wv_sb = res.tile([128, 6, DH], BF16)
        wp_sb = res.tile([128, 24, DIM], BF16)
        ebs_sb = res.tile([128, HEADS, 2, 256], BF16)
        bqk_r = res.tile([1, HEADS, 128], BF16)
        bv_r = res.tile([1, DH], BF16)
        pb_r = res.tile([1, DIM], BF16)
        one_r = res.tile([1, 256], BF16)     # rank-1 bias matmul rhs/lhsT
        six_k = res.tile([128, 128], BF16)   # 6.0: rowsum bcast to 128 parts
        b3 = res.tile([128, 1], F32)

        def load_x(b):
            xt = xio.tile([128, 6, 256], BF16, name=f"x{b}", tag="x")
            nc.sync.dma_start(xt[:], x_t.ap()[:, :, b * 256:(b + 1) * 256])
            return xt

        x_tiles = {0: load_x(0)}
        x_tiles[1] = load_x(1)
        nc.sync.dma_start(bqk_r[:], bqk.ap())
        for cot in range(12):
            nc.sync.dma_start(wqk_sb[:, :, cot * 128:(cot + 1) * 128],
                              wqk_t.ap()[cot])
        nc.gpsimd.dma_start(bv_r[:], bv.ap())
        for cob in range(6):
            nc.gpsimd.dma_start(wv_sb[:, :, cob * 512:(cob + 1) * 512],
                                wv_t.ap()[cob])
        for j in range(2):
            nc.scalar.dma_start(ebs_sb[:, :, j, :],
                                ebs.ap()[:, j].rearrange("h p n -> p h n"))
        nc.sync.dma_start(pb_r[:], pb.ap())
        nc.vector.memset(one_r[:], 1.0)
        nc.vector.memset(six_k[:], 6.0)
        nc.vector.memset(b3[:], 3.0)
        for dq in range(4):
            nc.scalar.dma_start(
                wp_sb[:, dq * 6:(dq + 1) * 6, :],
                wp_t.ap()[dq * 6:(dq + 1) * 6].rearrange("d p c -> p d c"))

        def qk_chunk(b, pr, qk2, xt):
            qps = qps_p.tile([128, 512], F32, name=f"qps{b}_{pr}",
                             tag="qps", bufs=2)
            for half in range(2):
                cot = 2 * pr + half
                csl = slice(half * 256, (half + 1) * 256)
                nc.tensor.matmul(qps[:, csl], bqk_r[:, cot, :], one_r[:],
                                 start=True, stop=False)
                for c in range(6):
                    nc.tensor.matmul(
                        qps[:, csl],
                        wqk_sb[:, c, cot * 128:(cot + 1) * 128],
                        xt[:, c, :], start=False, stop=(c == 5))
            nc.scalar.copy(qk2[:, pr, :], qps[:])

        def v_chunk(b, cob, tt, v_sb, xt):
            dsl = slice(cob * 512, (cob + 1) * 512)
            vps = vps_p.tile([128, 512], F32, name=f"vps{b}_{tt}_{cob}",
                             tag="vps", bufs=2)
            nc.tensor.matmul(vps[:], one_r[:, 0:128], bv_r[:, dsl],
                             start=True, stop=False)
            for c in range(6):
                nc.tensor.matmul(
                    vps[:], xt[:, c, tt * 128:(tt + 1) * 128],
                    wv_sb[:, c, dsl], start=False, stop=(c == 5))
            nc.vector.tensor_scalar_add(v_sb[:, tt, dsl], vps[:], 0.0)

        def emit_gemm_qk(b):
            xt = x_tiles[b]
            qk2 = qkio.tile([128, 6, 512], BF16, name=f"qk{b}", tag="qk2")
            for pr in range(6):
                qk_chunk(b, pr, qk2, xt)
            return qk2

        def emit_proj_chunk(h_te, bprev, tb, cc):
            # token-major: out[i, c] with h_t as lhsT, wp as moving operand
            w = 512 if cc == 0 else 256
            csl = slice(cc * 512, cc * 512 + w)
            py = py_p.tile([128, w], F32, name=f"py{bprev}_{tb}_{cc}",
                           tag="py", bufs=1)
            nc.tensor.matmul(py[:], one_r[:, 0:128], pb_r[:, csl],
                             start=True, stop=False)
            for dt_ in range(24):
                nc.tensor.matmul(
                    py[:], h_te[:, dt_, tb * 128:(tb + 1) * 128],
                    wp_sb[:, dt_, csl], start=False, stop=(dt_ == 23))
            yst = yio.tile([128, w], F32, name=f"yst{bprev}_{tb}_{cc}",
                           tag="yst", bufs=2)
            nc.vector.tensor_scalar_add(yst[:], py[:], 0.0)
            nc.sync.dma_start(y_t.ap()[2 * bprev + tb][:, csl], yst[:])

        gen = {}
        qk0 = (emit_gemm_qk(0),)
        qk1 = (emit_gemm_qk(1),)
        # v-GEMM for b0/b1 cob-interleaved so each wv column slice is
        # consumed for both batch elems as soon as it lands
        xt0, xt1 = x_tiles[0], x_tiles[1]
        v0 = vio.tile([128, 2, DH], BF16, name="v0", tag="v")
        v1 = vio.tile([128, 2, DH], BF16, name="v1", tag="v")
        for cob in range(6):
            dsl = slice(cob * 512, (cob + 1) * 512)
            for vt, xt in ((v0, xt0), (v1, xt1)):
                for tt in range(2):
                    vps = vps_p.tile([128, 512], F32,
                                     name=f"vpsP{vt.name}_{tt}_{cob}",
                                     tag="vps", bufs=2)
                    nc.tensor.matmul(vps[:], one_r[:, 0:128], bv_r[:, dsl],
                                     start=True, stop=False)
                    for c in range(6):
                        nc.tensor.matmul(
                            vps[:], xt[:, c, tt * 128:(tt + 1) * 128],
                            wv_sb[:, c, dsl], start=False, stop=(c == 5))
                    nc.vector.tensor_scalar_add(vt[:, tt, dsl], vps[:], 0.0)
        gen[0] = qk0 + (v0,)
        gen[1] = qk1 + (v1,)

        prev = None  # (h_t, b, yst) pending proj

        for b in range(BPC):
            if b + 2 < BPC:
                x_tiles[b + 2] = load_x(b + 2)
            qk2_sb, v_sb = gen.pop(b)
            h_t = hb.tile([128, 24, 256], BF16, name=f"ht{b}", tag="ht")

            state = {}

            def head_front(h):
                pr, off = h // 2, (h % 2) * 64
                s_ps = so_p.tile([128, 512], F32, name=f"s{b}_{h}", tag="so",
                                 bufs=2)
                for jt in range(2):
                    nc.tensor.matmul(
                        s_ps[:, jt * 256:(jt + 1) * 256],
                        qk2_sb[off:off + 64, pr,
                               256 + jt * 128:256 + (jt + 1) * 128],
                        qk2_sb[off:off + 64, pr, 0:256],
                        start=True, stop=True)
                es = work.tile([128, 512], BF16, name=f"es{b}_{h}", tag="es",
                               bufs=3)
                nc.scalar.activation(es[:], s_ps[:], EXP)
                nc.gpsimd.tensor_tensor(es[:], es[:], ebs_sb[:, h], MULT)
                state[h] = es

            def head_mid(h):
                es = state[h]
                rs = rs_p.tile([128, 256], F32, name=f"rs{b}_{h}", tag="rs",
                               bufs=1)
                for jt in range(2):
                    nc.tensor.matmul(
                        rs[:], six_k[:], es[:, jt * 256:(jt + 1) * 256],
                        start=(jt == 0), stop=(jt == 1))
                o_ps = so_p.tile([128, 512], F32, name=f"o{b}_{h}", tag="so",
                                 bufs=2)
                for dvt in range(2):
                    for jt in range(2):
                        nc.tensor.matmul(
                            o_ps[:, dvt * 256:(dvt + 1) * 256],
                            v_sb[:, jt, h * 256 + dvt * 128:
                                 h * 256 + (dvt + 1) * 128],
                            es[:, jt * 256:(jt + 1) * 256],
                            start=(jt == 0), stop=(jt == 1))
                rcp1 = work.tile([128, 256], F32, name=f"rcp{b}_{h}",
                                 tag="rcp", bufs=2)
                nc.vector.reciprocal(rcp1[:], rs[:])
                state[h] = (o_ps, rcp1)

            def head_tail(h):
                o_ps, rcp1 = state.pop(h)
                r_ap = rcp1[:]
                rdup = bass.AP(tensor=r_ap.tensor, offset=r_ap.offset,
                               ap=[list(r_ap.ap[0]), [0, 2]]
                               + [list(p) for p in r_ap.ap[1:]])
                y0 = work.tile([128, 512], BF16, name=f"y0{b}_{h}", tag="y0",
                               bufs=3)
                nc.vector.tensor_tensor(y0[:], o_ps[:], rdup, MULT)
                a = work.tile([128, 512], BF16, name=f"a{b}_{h}", tag="a",
                              bufs=2)
                nc.scalar.activation(a[:], y0[:], RELU, bias=b3[:], scale=6.0)
                nc.gpsimd.tensor_scalar_min(a[:], a[:], 6.0)
                nc.gpsimd.tensor_tensor(
                    h_t[:, 2 * h:2 * h + 2, :], y0[:], a[:], MULT)

            proj_steps = {3: (0, 0), 6: (0, 1), 9: (1, 0), 11: (1, 1)}
            for h in range(HEADS):
                head_front(h)
                if h in proj_steps and prev is not None:
                    emit_proj_chunk(prev[0], prev[1], *proj_steps[h])
                if h >= 1:
                    head_mid(h - 1)
                if h >= 2:
                    head_tail(h - 2)
            head_mid(HEADS - 1)
            head_tail(HEADS - 2)
            head_tail(HEADS - 1)

            if b + 2 < BPC:
                xt2 = x_tiles[b + 2]
                qk2n = emit_gemm_qk(b + 2)
                vn = vio.tile([128, 2, DH], BF16, name=f"v{b + 2}", tag="v")
                for cob in range(6):
                    for tt in range(2):
                        v_chunk(b + 2, cob, tt, vn, xt2)
                gen[b + 2] = (qk2n, vn)
                del x_tiles[b + 2]
            prev = (h_t, b)

        for tb in range(2):
            for cc in range(2):
                emit_proj_chunk(prev[0], prev[1], tb, cc)


def _build(reps=1, unroll=1):
    nc = bacc.Bacc("TRN2", target_bir_lowering=False, debug=False)
    x_t = nc.dram_tensor("x_t", [128, 6, T], BF16, kind="ExternalInput")
    wqk_t = nc.dram_tensor("wqk_t", [12, 128, 6, 128], BF16,
                           kind="ExternalInput")
    wv_t = nc.dram_tensor("wv_t", [6, 128, 6, 512], BF16, kind="ExternalInput")
    wp_t = nc.dram_tensor("wp_t", [24, 128, DIM], BF16, kind="ExternalInput")
    bqk = nc.dram_tensor("bqk", [1, HEADS, 128], BF16, kind="ExternalInput")
    bv = nc.dram_tensor("bv", [DH], BF16, kind="ExternalInput")
    ebs = nc.dram_tensor("ebs", [HEADS, 2, 128, 256], BF16,
                         kind="ExternalInput")
    pb = nc.dram_tensor("pb", [1, DIM], BF16, kind="ExternalInput")
    y_t = nc.dram_tensor("y_t", [16, 128, DIM], F32, kind="ExternalOutput")

    with tile.TileContext(nc) as tc:
        if unroll > 1:
            for _ in range(unroll):
                _emit(tc, nc, x_t, wqk_t, wv_t, wp_t, bqk, bv, ebs, pb, y_t)
        elif reps == 1:
            _emit(tc, nc, x_t, wqk_t, wv_t, wp_t, bqk, bv, ebs, pb, y_t)
        else:
            with tc.For_i(0, reps, 1):
                _emit(tc, nc, x_t, wqk_t, wv_t, wp_t, bqk, bv, ebs, pb,
                      y_t)
    nc.compile()
    return nc


_NC = None


def _get_nc():
    global _NC
    if _NC is None:
        _NC = _build()
    return _NC


def _prep_host(qkv_w, qkv_gamma, qkv_beta, qkv_mean, qkv_var,
               attn_biases, proj_w, proj_gamma, proj_beta, proj_mean,
               proj_var, bias_idxs):
    f32 = np.float32
    bf16 = ml_dtypes.bfloat16
    qkv_w = np.asarray(qkv_w, f32)
    s = np.asarray(qkv_gamma, f32) / np.sqrt(np.asarray(qkv_var, f32) + EPS)
    Wf = qkv_w * s[:, None]
    bf_ = np.asarray(qkv_beta, f32) - np.asarray(qkv_mean, f32) * s

    base = np.arange(HEADS, dtype=np.int64)[:, None] * 384
    qk_ch = (base + np.arange(128)[None, :]).reshape(-1)
    v_ch = (base + 128 + np.arange(256)[None, :]).reshape(-1)

    Wqk = Wf[qk_ch].copy()
    bqk_v = bf_[qk_ch].copy()
    Wqk.reshape(HEADS, 128, DIM)[:, :64, :] *= SCALE
    bqk_v.reshape(HEADS, 128)[:, :64] *= SCALE

    # reorder rows into head-pair groups: [q_h0|q_h1], [k_h0|k_h1], ...
    wq = Wqk.reshape(6, 2, 2, 64, DIM).transpose(0, 2, 1, 3, 4)
    bq = bqk_v.reshape(6, 2, 2, 64).transpose(0, 2, 1, 3)
    Wqk2 = wq.reshape(1536, DIM)
    bqk2 = bq.reshape(1536)

    wqk_t = np.ascontiguousarray(
        Wqk2.T.reshape(6, 128, 12, 128).transpose(2, 1, 0, 3)).astype(bf16)
    bqk_np = bqk2.reshape(1, HEADS, 128).astype(bf16)
    wv_t = np.ascontiguousarray(
        Wf[v_ch].T.reshape(6, 128, 6, 512).transpose(2, 1, 0, 3)).astype(bf16)
    bv_np = bf_[v_ch].astype(bf16)

    proj_w = np.asarray(proj_w, f32)
    sp = np.asarray(proj_gamma, f32) / np.sqrt(np.asarray(proj_var, f32) + EPS)
    bp_v = np.asarray(proj_beta, f32) - np.asarray(proj_mean, f32) * sp
    wp_t = np.ascontiguousarray(
        proj_w.T * sp[None, :]).reshape(24, 128, DIM).astype(bf16)
    pb_np = bp_v.reshape(1, DIM).astype(bf16)

    bias_full = np.asarray(attn_biases, f32)[:, np.asarray(bias_idxs)]
    ebs_np = np.ascontiguousarray(
        np.exp(bias_full).transpose(0, 2, 1)).reshape(
        HEADS, 2, 128, 256).astype(bf16)

    return dict(wqk_t=wqk_t, wv_t=wv_t, bqk=bqk_np, bv=bv_np,
                wp_t=wp_t, pb=pb_np, ebs=ebs_np)


def kernel(x, qkv_w, qkv_gamma, qkv_beta, qkv_mean, qkv_var,
           attn_biases, proj_w, proj_gamma, proj_beta, proj_mean, proj_var,
           bias_idxs):
    x = np.asarray(x, np.float32)
    shared = _prep_host(qkv_w, qkv_gamma, qkv_beta, qkv_mean, qkv_var,
                        attn_biases, proj_w, proj_gamma, proj_beta,
                        proj_mean, proj_var, bias_idxs)
    in_maps = []
    for ci in range(NCORES):
        xc = x[ci * BPC:(ci + 1) * BPC].reshape(T, DIM)
        x_tc = np.ascontiguousarray(
            xc.T.reshape(6, 128, T).transpose(1, 0, 2)).astype(
            ml_dtypes.bfloat16)
        m = dict(shared)
        m["x_t"] = x_tc
        in_maps.append(m)

    nc = _get_nc()
    res = run_bass_kernel_spmd(nc, in_maps, core_ids=list(range(NCORES)))

    out = np.empty((B, SEQ, DIM), np.float32)
    for ci in range(NCORES):
        yt = np.asarray(res.results[ci]["y_t"]).reshape(T, DIM)
        out[ci * BPC:(ci + 1) * BPC] = yt.reshape(BPC, SEQ, DIM)
    return out
